# revision 29
# baseline (speedup 1.0000x reference)
"""FISTA compressed-sensing kernel for Trainium2 (8 NeuronCores, SPMD).

Problem: for each of 64 patches (x3 channels), run 200 FISTA iterations of
    min_x 0.5||A x - b||^2 + lam||x||_1,   A: (81, 5184)
Sharding: pure data-parallel over the batch -- 8 patches x 3 channels = 24
columns per core; A replicated.

Per-core formulation (column matrix Y: (5184, 24)):
    Ay   = A @ Y                      (81, 24)
    G    = A^T @ Ay - Atb             (5184, 24)   [-Atb^T folded host-side
                                                    into 24 extra contraction
                                                    rows of the A^T weights]
    Z    = Y - mu*G
    Xn   = soft_threshold(Z, lam*mu) = Z - clamp(Z, -thr, thr)
    Y'   = Xn + coef_i * (Xn - X)

Layout: D=5184 padded to 5248 = 41*128; state tiles [128, kt, 24] with
d = kt*128 + p. Column groups g0 = ktiles 0..20, g1 = 21..40 so each
gradient group fits one PSUM bank.

Precision: fp32 matmuls on TRN2 self-load weights at ~2.7 ns/column
(measured), so reloading all of A twice per iteration costs ~23 us.
bf16 matmul pairs measured ~8x faster -- but plain bf16 diverges from the
reference trajectory (0.9 rel err at 200 iters; the LASSO fixed point is
extremely sensitive to A perturbation). Solution: split precision.
A = A_hi + A_lo (both bf16), y = y_hi + y_lo, and each product uses three
bf16 terms  A_hi@y_hi + A_hi@y_lo + A_lo@y_hi  accumulated in fp32 PSUM
(~2^-16 effective mantissa; emulated end-to-end rel err 2.6e-3 at 200
iters). All elementwise state math stays fp32.

The 200 iterations run in a hardware For_i loop (2 FISTA steps per body so
the x ping-pong is static; per-step momentum coefficient read from an SBUF
table indexed by the loop var), keeping the NEFF size independent of the
iteration count. The PJRT executable is jitted ONCE per build and cached;
input-independent operands stay device-resident, so a steady-state call
uploads only the per-core -Atb^T blocks.
"""

import os

import numpy as np

import concourse.bass as bass
import concourse.mybir as mybir
import concourse.tile as tile
from concourse.bass import ds

F32 = mybir.dt.float32
BF16 = mybir.dt.bfloat16

M = 81            # measurements (9x9 camera patch)
D = 5184          # atoms (72x72 upsampled grid)
KT = 41           # 128-row tiles covering D (padded to 5248)
DP = KT * 128     # 5248
NCORES = 8
B = 64
BPC = B // NCORES           # 8 patches per core
N = BPC * 3                 # 24 state columns per core
ITERS = int(os.environ.get("FISTA_ITERS", "200"))
CT0 = 96                    # partition row where the -Atb^T block starts
G0, G1 = 21, 20             # ktiles per column group (504 / 480 psum cols)

_RUNNERS = {}


def _legalize_waits(nc):
    """This walrus build accepts at most ONE semaphore wait per instruction
    (setupSyncWait: 'Too many sync wait commands'). Tile emits multi-wait
    instructions; split the excess waits onto injected same-engine NoOps
    placed immediately before the instruction (engine queues are FIFO, so
    semantics are identical)."""
    n = 0
    for fn in nc.m.functions:
        for bb in fn.blocks:
            insts = bb.instructions
            out = []
            changed = False
            for ins in insts:
                si = ins.sync_info
                ow = list(si.on_wait) if si is not None else []
                if len(ow) > 1 and ins.engine is not None:
                    for w in ow[:-1]:
                        n += 1
                        out.append(mybir.InstNoOp(
                            name=f"I-waitnop-{n}",
                            engine=ins.engine,
                            ins=[],
                            outs=[],
                            debug=ins.debug,
                            sync_info=mybir.SyncInfo(on_wait=[w], on_update=[]),
                        ))
                    ins.sync_info = mybir.SyncInfo(
                        on_wait=[ow[-1]], on_update=list(si.on_update))
                    changed = True
                out.append(ins)
            if changed:
                bb.instructions = out
    return n


def _fista_coefs(iters):
    t = 1.0
    coefs = []
    for _ in range(iters):
        t_new = (1.0 + float(np.sqrt(1.0 + 4.0 * t * t))) / 2.0
        coefs.append((t - 1.0) / t_new)
        t = t_new
    return coefs


def _build(mu_s, thr, iters, unroll=2, hint=True, staggered=False,
           ablate=None, chain="dve"):
    """Build the Bass module (same program for all 8 cores).

    unroll: FISTA steps per For_i body (even, divides iters).
    ablate: None | 'mm_only' | 'ew_only' (timing experiments).
    chain: engine placement for the post-gradient elementwise chain:
      'dve' (all DVE) | 'pool' (clamp/sub on Pool) | 'mixed' (g0 on DVE,
      g1 on Pool).
    """
    if iters < unroll:
        unroll = iters
    assert unroll % 2 == 0 and iters % unroll == 0
    half = iters // unroll
    nc = bass.Bass()

    at_hi_d = nc.declare_dram_parameter("at_hi", [128, KT, M], BF16,
                                        isOutput=False)
    at_lo_d = nc.declare_dram_parameter("at_lo", [128, KT, M], BF16,
                                        isOutput=False)
    a_hi_d = nc.declare_dram_parameter("a_hi", [128, KT, 128], BF16,
                                       isOutput=False)
    a_lo_d = nc.declare_dram_parameter("a_lo", [128, KT, 128], BF16,
                                       isOutput=False)
    nat_hi_d = nc.declare_dram_parameter("natbt_hi", [N, KT, 128], BF16,
                                         isOutput=False)
    nat_lo_d = nc.declare_dram_parameter("natbt_lo", [N, KT, 128], BF16,
                                         isOutput=False)
    ay0_d = nc.declare_dram_parameter("ayinit", [128, 2, N], BF16,
                                      isOutput=False)
    cf_d = nc.declare_dram_parameter("coefs", [128, unroll, half], F32,
                                     isOutput=False)
    xout_d = nc.declare_dram_parameter("xout", [128, KT, N], F32,
                                       isOutput=True)

    with tile.TileContext(nc) as tc:
        with (
            tc.tile_pool(name="weights", bufs=1) as wpool,
            tc.tile_pool(name="state", bufs=1) as spool,
            tc.tile_pool(name="tmp", bufs=2) as tpool,
            tc.tile_pool(name="psum_ay", bufs=2, space="PSUM") as ppool_ay,
            tc.tile_pool(name="psum_gp", bufs=1, space="PSUM") as ppool_gp,
        ):
            # --- persistent SBUF tensors -------------------------------
            at_hi = wpool.tile([128, KT, M], BF16)    # lhsT for matmul1
            at_lo = wpool.tile([128, KT, M], BF16)
            w2_hi = wpool.tile([128, KT, 128], BF16)  # lhsT for matmul2
            w2_lo = wpool.tile([128, KT, 128], BF16)
            # ay hi/lo pair packed in one tile: [:, 0, :] = hi, [:, 1, :] = lo
            # (one 48-col AP so matmul2 consumes both halves in one pass)
            ayp = wpool.tile([128, 2, N], BF16)
            cf_sb = wpool.tile([128, unroll, half], F32)  # momentum coefs
            # y kept fp32 plus its bf16 hi/lo split packed [128, G, 2, N]
            y_sb = [spool.tile([128, G0, N], F32, tag="y0", name="y0"),
                    spool.tile([128, G1, N], F32, tag="y1", name="y1")]
            yp_sb = [spool.tile([128, G0, 2, N], BF16, tag="yp0", name="yp0"),
                     spool.tile([128, G1, 2, N], BF16, tag="yp1", name="yp1")]
            # x ping-pong: x_sb[s][g]
            x_sb = [[spool.tile([128, G0, N], F32, tag=f"x{s}0", name=f"x{s}0"),
                     spool.tile([128, G1, N], F32, tag=f"x{s}1", name=f"x{s}1")]
                    for s in range(2)]

            nc.sync.dma_start(out=at_hi[:], in_=at_hi_d[:])
            nc.sync.dma_start(out=at_lo[:], in_=at_lo_d[:])
            nc.sync.dma_start(out=w2_hi[:], in_=a_hi_d[:])
            nc.sync.dma_start(out=w2_lo[:], in_=a_lo_d[:])
            # overwrite contraction rows CT0..CT0+N with the -Atb^T blocks
            nc.sync.dma_start(out=w2_hi[CT0:CT0 + N, :, :], in_=nat_hi_d[:])
            nc.sync.dma_start(out=w2_lo[CT0:CT0 + N, :, :], in_=nat_lo_d[:])
            nc.sync.dma_start(out=ayp[:], in_=ay0_d[:])
            nc.sync.dma_start(out=cf_sb[:], in_=cf_d[:])

            # initial state: x_prev = y = 0
            for g in range(2):
                nc.vector.memset(y_sb[g][:], 0.0)
                nc.vector.memset(yp_sb[g][:], 0.0)
                nc.vector.memset(x_sb[1][g][:], 0.0)

            # group g -> (ktile offset, ktile count)
            gidx = [(0, G0), (G0, G1)]

            def fista_step(cur, prev, coef_ap):
                do_mm = ablate != 'ew_only'
                do_ew = ablate != 'mm_only'
                # matmul1: Ay = A @ Y -> psum (81, 2*24): per ktile one
                # 48-col pass A_hi @ [y_hi|y_lo], plus A_lo @ y_hi
                # accumulated onto the left half. halves sum to the 3-term
                # split product.
                if do_mm:
                    ay_ps = ppool_ay.tile([M, 2, N], F32, tag="ay")
                    for kt in range(KT):
                        g, j = (0, kt) if kt < G0 else (1, kt - G0)
                        nc.tensor.matmul(
                            ay_ps[:],
                            at_hi[:, kt, :],
                            yp_sb[g][:, j, :, :],
                            start=(kt == 0),
                            stop=False,
                            skip_group_check=True,
                        )
                        nc.tensor.matmul(
                            ay_ps[:, 0, :],
                            at_lo[:, kt, :],
                            yp_sb[g][:, j, 0, :],
                            start=False,
                            stop=(kt == KT - 1),
                            skip_group_check=True,
                        )
                    # ay = left + right half (one PSUM operand per DVE op);
                    # split into bf16 hi + lo (identity rows CT0.. of ayp
                    # stay from init)
                    ayt = tpool.tile([M, N], F32, tag="ayt")
                    ays = tpool.tile([M, N], F32, tag="ays")
                    nc.vector.tensor_copy(ayt[:], ay_ps[:, 0, :])
                    nc.vector.scalar_tensor_tensor(
                        out=ays[:], in0=ay_ps[:, 1, :], scalar=1.0,
                        in1=ayt[:], op0=mybir.AluOpType.mult,
                        op1=mybir.AluOpType.add)
                    nc.vector.tensor_copy(ayp[0:M, 0, :], ays[:])
                    nc.vector.tensor_sub(ayp[0:M, 1, :], ays[:],
                                         ayp[0:M, 0, :])

                # matmul2 + elementwise, per column group. Gradient PSUM is
                # chunked into <=10-ktile tiles so a 48-col matmul output
                # never crosses a PSUM bank (512 fp32).
                for g in range(2):
                    k0, ng = gidx[g]
                    chunks = [(c0, min(10, ng - c0))
                              for c0 in range(0, ng, 10)]
                    z = tpool.tile([128, ng, N], F32, tag=f"z{g}")
                    if do_mm:
                        for c0, nch in chunks:
                            gp = ppool_gp.tile([128, nch, 2, N], F32,
                                               tag=f"gp{g}_{c0}",
                                               name=f"gp{g}_{c0}")
                            for j in range(nch):
                                nc.tensor.matmul(
                                    gp[:, j, :, :],
                                    w2_hi[:, k0 + c0 + j, :],
                                    ayp[:],
                                    start=True,
                                    stop=False,
                                    skip_group_check=True,
                                )
                                nc.tensor.matmul(
                                    gp[:, j, 0, :],
                                    w2_lo[:, k0 + c0 + j, :],
                                    ayp[:, 0, :],
                                    start=False,
                                    stop=True,
                                    skip_group_check=True,
                                )
                            # z = y - mu*(gpL + gpR): two chained stt ops,
                            # each reading one PSUM half
                            zc = z[:, c0:c0 + nch, :]
                            nc.vector.scalar_tensor_tensor(
                                out=zc, in0=gp[:, :, 0, :], scalar=-mu_s,
                                in1=y_sb[g][:, c0:c0 + nch, :],
                                op0=mybir.AluOpType.mult,
                                op1=mybir.AluOpType.add)
                            nc.vector.scalar_tensor_tensor(
                                out=zc, in0=gp[:, :, 1, :], scalar=-mu_s,
                                in1=zc, op0=mybir.AluOpType.mult,
                                op1=mybir.AluOpType.add)
                    else:
                        nc.vector.tensor_scalar_mul(z[:], y_sb[g][:], 0.5)
                    if chain == "pool" or (chain == "mixed" and g == 1):
                        eng = nc.gpsimd
                    else:
                        eng = nc.vector
                    # soft threshold: xn = z - clamp(z, -thr, thr)
                    c = tpool.tile([128, ng, N], F32, tag=f"c{g}")
                    eng.tensor_scalar(
                        out=c[:],
                        in0=z[:],
                        scalar1=thr,
                        scalar2=-thr,
                        op0=mybir.AluOpType.min,
                        op1=mybir.AluOpType.max,
                    )
                    xn = x_sb[cur][g]
                    eng.tensor_sub(xn[:], z[:], c[:])
                    # momentum: y = xn + coef*(xn - x_prev), then split y
                    d = tpool.tile([128, ng, N], F32, tag=f"d{g}")
                    eng.tensor_sub(d[:], xn[:], x_sb[prev][g][:])
                    eng.scalar_tensor_tensor(
                        out=y_sb[g][:],
                        in0=d[:],
                        scalar=coef_ap,
                        in1=xn[:],
                        op0=mybir.AluOpType.mult,
                        op1=mybir.AluOpType.add,
                    )
                    eng.tensor_copy(yp_sb[g][:, :, 0, :], y_sb[g][:])
                    eng.tensor_sub(yp_sb[g][:, :, 1, :], y_sb[g][:],
                                   yp_sb[g][:, :, 0, :])

            # --- FISTA iterations: hardware loop, `unroll` steps/body --
            with tc.For_i(0, half, 1,
                          hint_engines=((mybir.EngineType.PE,) if hint
                                        else ()),
                          staggered_reset=staggered) as it:
                for s in range(unroll):
                    fista_step(s % 2, (s + 1) % 2, cf_sb[:, s, ds(it, 1)])

            # --- write back final x ------------------------------------
            nc.sync.dma_start(out=xout_d[:, 0:G0, :], in_=x_sb[1][0][:])
            nc.sync.dma_start(out=xout_d[:, G0:KT, :], in_=x_sb[1][1][:])

    _legalize_waits(nc)
    return nc


def _split16(x):
    bf = mybir.dt.np(BF16)
    hi = x.astype(bf)
    lo = (x - hi.astype(np.float32)).astype(bf)
    return hi, lo


def _const_inputs(A, iters, unroll=2):
    """Input-independent operands: A^T tiles, A tiles, ay init, coefs."""
    if iters < unroll:
        unroll = iters
    A = np.asarray(A, np.float32)
    A_pad = np.zeros((M, DP), np.float32)
    A_pad[:, :D] = A
    a_tiles = np.zeros((128, KT, 128), np.float32)
    a_tiles[:M] = A_pad.reshape(M, KT, 128)
    ay_init = np.zeros((128, 2, N), np.float32)
    ay_init[CT0 : CT0 + N, 0] = np.eye(N, dtype=np.float32)
    at_tiles = np.ascontiguousarray(
        A_pad.T.reshape(KT, 128, M).transpose(1, 0, 2))  # [128, KT, M]

    coefs = _fista_coefs(iters)
    half = iters // unroll
    cf = np.zeros((unroll, half), np.float32)
    for s in range(unroll):
        cf[s] = coefs[s::unroll]
    cf_tab = np.ascontiguousarray(
        np.broadcast_to(cf[None], (128, unroll, half)).astype(np.float32))

    at_hi, at_lo = _split16(at_tiles)
    a_hi, a_lo = _split16(a_tiles)
    return {"at_hi": at_hi, "at_lo": at_lo, "a_hi": a_hi, "a_lo": a_lo,
            "ayinit": ay_init.astype(mybir.dt.np(BF16)),
            "coefs": cf_tab}, A_pad


def _natbt_inputs(inp, A_pad):
    """Per-core -Atb^T blocks (bf16 hi/lo), shaped [N, KT, 128]."""
    inp = np.asarray(inp, np.float32)
    his, los = [], []
    for c in range(NCORES):
        chunk = inp[c * BPC : (c + 1) * BPC]            # (8, 81, 3)
        b_mat = chunk.transpose(1, 0, 2).reshape(M, N)  # (81, 24)
        ct = b_mat.T @ A_pad                            # (24, 5248) = (Atb)^T
        hi, lo = _split16(np.ascontiguousarray(-ct.reshape(N, KT, 128)))
        his.append(hi)
        los.append(lo)
    return {"natbt_hi": his, "natbt_lo": los}


def _prep_inputs(inp, A, iters=None, unroll=2):
    """Per-core input maps (kept for compatibility with direct
    run_bass_kernel_spmd invocations, e.g. simulation)."""
    if iters is None:
        iters = ITERS
    const, A_pad = _const_inputs(A, iters, unroll)
    nat = _natbt_inputs(inp, A_pad)
    return [dict(const, natbt_hi=nat["natbt_hi"][c],
                 natbt_lo=nat["natbt_lo"][c]) for c in range(NCORES)]


class _Runner:
    """PJRT executable for one built module, jitted once. Input-independent
    operands live on device; per-call we upload only the -Atb^T blocks."""

    def __init__(self, nc, A, iters, unroll=2):
        import jax
        from jax.sharding import Mesh, PartitionSpec, NamedSharding
        from jax.experimental.shard_map import shard_map
        from concourse import bass2jax

        bass2jax.install_neuronx_cc_hook()
        self.nc = nc
        self.const, self.A_pad = _const_inputs(A, iters, unroll)

        assert nc.dbg_addr is None or not nc.dbg_callbacks
        extra = {}
        if nc.dbg_addr is not None:
            extra[nc.dbg_addr.name] = np.zeros((1, 2), np.uint32)

        partition_name = (nc.partition_id_tensor.name
                          if nc.partition_id_tensor else None)
        in_names, out_names, out_avals, zero_outs = [], [], [], []
        self.in_dtypes = {}
        for alloc in nc.m.functions[0].allocations:
            if not isinstance(alloc, mybir.MemoryLocationSet):
                continue
            name = alloc.memorylocations[0].name
            if alloc.kind == "ExternalInput":
                if name != partition_name:
                    in_names.append(name)
                    self.in_dtypes[name] = mybir.dt.np(alloc.dtype)
            elif alloc.kind == "ExternalOutput":
                shape = tuple(alloc.tensor_shape)
                dtype = mybir.dt.np(alloc.dtype)
                out_names.append(name)
                out_avals.append(jax.core.ShapedArray(shape, dtype))
                zero_outs.append(np.zeros(shape, dtype))
        n_params = len(in_names)
        n_outs = len(out_names)
        all_names = in_names + out_names
        if partition_name is not None:
            all_names.append(partition_name)

        def _body(*args):
            operands = list(args)
            if partition_name is not None:
                operands.append(bass2jax.partition_id_tensor())
            outs = bass2jax._bass_exec_p.bind(
                *operands,
                out_avals=tuple(out_avals),
                in_names=tuple(all_names),
                out_names=tuple(out_names),
                lowering_input_output_aliases=(),
                sim_require_finite=True,
                sim_require_nnan=True,
                nc=nc,
            )
            return tuple(outs)

        devices = jax.devices()[:NCORES]
        assert len(devices) == NCORES
        self.mesh = Mesh(np.asarray(devices), ("core",))
        in_specs = (PartitionSpec("core"),) * (n_params + n_outs)
        out_specs = (PartitionSpec("core"),) * n_outs
        donate = tuple(range(n_params, n_params + n_outs))
        self.fn = jax.jit(
            shard_map(_body, mesh=self.mesh, in_specs=in_specs,
                      out_specs=out_specs, check_rep=False),
            donate_argnums=donate, keep_unused=True)

        self.in_names = in_names
        self.out_names = out_names
        self.out_avals = out_avals
        self.zero_shapes = [(z.shape, z.dtype) for z in zero_outs]

        # Device-resident constant inputs (replicated per core, concat on
        # axis 0 as shard_map expects). extra covers dbg_addr if present.
        sharding = NamedSharding(self.mesh, PartitionSpec("core"))
        self.dev_const = {}
        for name in in_names:
            if name.startswith("natbt"):
                continue
            arr = self.const.get(name)
            if arr is None:
                arr = extra[name]
            glob = np.concatenate([arr] * NCORES, axis=0).astype(
                self.in_dtypes[name])
            self.dev_const[name] = jax.device_put(glob, sharding)

    def run(self, inp):
        nat = _natbt_inputs(inp, self.A_pad)
        args = []
        for name in self.in_names:
            if name.startswith("natbt"):
                args.append(np.concatenate(nat[name], axis=0).astype(
                    self.in_dtypes[name]))
            else:
                args.append(self.dev_const[name])
        for shape, dtype in self.zero_shapes:
            args.append(np.zeros((NCORES * shape[0], *shape[1:]), dtype))
        out_arrs = self.fn(*args)
        res = []
        for c in range(NCORES):
            res.append({
                name: np.asarray(out_arrs[i]).reshape(
                    NCORES, *self.out_avals[i].shape)[c]
                for i, name in enumerate(self.out_names)})
        return res


def _unshard(results):
    outs = []
    for c in range(NCORES):
        xo = np.asarray(results[c]["xout"])              # [128, KT, N]
        x_dn = xo.transpose(1, 0, 2).reshape(DP, N)[:D]  # (5184, 24)
        outs.append(x_dn.reshape(72, 72, BPC, 3).transpose(2, 0, 1, 3))
    return np.concatenate(outs, 0).astype(np.float32)    # (64, 72, 72, 3)


def _get_runner(A, mu_s, thr, iters):
    key = (mu_s, thr, iters, hash(np.asarray(A, np.float32).tobytes()))
    if key not in _RUNNERS:
        nc = _build(mu_s, thr, iters)
        _RUNNERS[key] = _Runner(nc, A, iters)
    return _RUNNERS[key]


def _run(inp, A, lam, mu, trace=False):
    mu_s = float(np.asarray(mu).reshape(-1)[0])
    thr = float(np.asarray(lam).reshape(-1)[0]) * mu_s
    runner = _get_runner(A, mu_s, thr, ITERS)
    results = runner.run(inp)
    return _unshard(results), results


def kernel(inp, A, lam, mu):
    out, _ = _run(inp, A, lam, mu)
    return out


# revision 31
# speedup vs baseline: 8.4814x; 8.4814x over previous
"""FISTA compressed-sensing kernel for Trainium2 (8 NeuronCores, SPMD).

Problem: for each of 64 patches (x3 channels), run 200 FISTA iterations of
    min_x 0.5||A x - b||^2 + lam||x||_1,   A: (81, 5184)
Sharding: pure data-parallel over the batch -- 8 patches x 3 channels = 24
columns per core; A replicated.

Per-core formulation (column matrix Y: (5184, 24)):
    Ay   = A @ Y                      (81, 24)
    G    = A^T @ Ay - Atb             (5184, 24)   [-Atb^T folded host-side
                                                    into 24 extra contraction
                                                    rows of the A^T weights]
    Z    = Y - mu*G
    Xn   = soft_threshold(Z, lam*mu) = Z - clamp(Z, -thr, thr)
    Y'   = Xn + coef_i * (Xn - X)

Layout: D=5184 padded to 5248 = 41*128; state tiles [128, kt, 24] with
d = kt*128 + p. Column groups g0 = ktiles 0..20, g1 = 21..40 so each
gradient group fits one PSUM bank.

Precision: fp32 matmuls on TRN2 self-load weights at ~2.7 ns/column
(measured), so reloading all of A twice per iteration costs ~23 us.
bf16 matmul pairs measured ~8x faster -- but plain bf16 diverges from the
reference trajectory (0.9 rel err at 200 iters; the LASSO fixed point is
extremely sensitive to A perturbation). Solution: split precision.
A = A_hi + A_lo (both bf16), y = y_hi + y_lo, and each product uses three
bf16 terms  A_hi@y_hi + A_hi@y_lo + A_lo@y_hi  accumulated in fp32 PSUM
(~2^-16 effective mantissa; emulated end-to-end rel err 2.6e-3 at 200
iters). All elementwise state math stays fp32.

The 200 iterations run in a hardware For_i loop (2 FISTA steps per body so
the x ping-pong is static; per-step momentum coefficient read from an SBUF
table indexed by the loop var), keeping the NEFF size independent of the
iteration count. The PJRT executable is jitted ONCE per build and cached;
input-independent operands stay device-resident, so a steady-state call
uploads only the per-core -Atb^T blocks.
"""

import os

import numpy as np

import concourse.bass as bass
import concourse.mybir as mybir
import concourse.tile as tile
from concourse.bass import ds

F32 = mybir.dt.float32
BF16 = mybir.dt.bfloat16

M = 81            # measurements (9x9 camera patch)
D = 5184          # atoms (72x72 upsampled grid)
KT = 41           # 128-row tiles covering D (padded to 5248)
DP = KT * 128     # 5248
NCORES = 8
B = 64
BPC = B // NCORES           # 8 patches per core
N = BPC * 3                 # 24 state columns per core
ITERS = int(os.environ.get("FISTA_ITERS", "200"))
CT0 = 96                    # partition row where the -Atb^T block starts
G0, G1 = 21, 20             # ktiles per column group (504 / 480 psum cols)

_RUNNERS = {}


def _legalize_waits(nc):
    """This walrus build accepts at most ONE semaphore wait per instruction
    (setupSyncWait: 'Too many sync wait commands'). Tile emits multi-wait
    instructions; split the excess waits onto injected same-engine NoOps
    placed immediately before the instruction (engine queues are FIFO, so
    semantics are identical)."""
    n = 0
    for fn in nc.m.functions:
        for bb in fn.blocks:
            insts = bb.instructions
            out = []
            changed = False
            for ins in insts:
                si = ins.sync_info
                ow = list(si.on_wait) if si is not None else []
                if len(ow) > 1 and ins.engine is not None:
                    for w in ow[:-1]:
                        n += 1
                        out.append(mybir.InstNoOp(
                            name=f"I-waitnop-{n}",
                            engine=ins.engine,
                            ins=[],
                            outs=[],
                            debug=ins.debug,
                            sync_info=mybir.SyncInfo(on_wait=[w], on_update=[]),
                        ))
                    ins.sync_info = mybir.SyncInfo(
                        on_wait=[ow[-1]], on_update=list(si.on_update))
                    changed = True
                out.append(ins)
            if changed:
                bb.instructions = out
    return n


def _fista_coefs(iters):
    t = 1.0
    coefs = []
    for _ in range(iters):
        t_new = (1.0 + float(np.sqrt(1.0 + 4.0 * t * t))) / 2.0
        coefs.append((t - 1.0) / t_new)
        t = t_new
    return coefs


def _build(mu_s, thr, iters, unroll=2, hint=True, staggered=False,
           ablate=None, chain="dve"):
    """Build the Bass module (same program for all 8 cores).

    unroll: FISTA steps per For_i body (even, divides iters).
    ablate: None | 'mm_only' | 'ew_only' (timing experiments).
    chain: engine placement for the post-gradient elementwise chain:
      'dve' (all DVE) | 'pool' (clamp/sub on Pool) | 'mixed' (g0 on DVE,
      g1 on Pool).
    """
    if iters < unroll:
        unroll = iters
    assert unroll % 2 == 0 and iters % unroll == 0
    half = iters // unroll
    nc = bass.Bass()

    at_hi_d = nc.declare_dram_parameter("at_hi", [128, KT, M], BF16,
                                        isOutput=False)
    at_lo_d = nc.declare_dram_parameter("at_lo", [128, KT, M], BF16,
                                        isOutput=False)
    a_hi_d = nc.declare_dram_parameter("a_hi", [128, KT, 128], BF16,
                                       isOutput=False)
    a_lo_d = nc.declare_dram_parameter("a_lo", [128, KT, 128], BF16,
                                       isOutput=False)
    nat_hi_d = nc.declare_dram_parameter("natbt_hi", [N, KT, 128], BF16,
                                         isOutput=False)
    nat_lo_d = nc.declare_dram_parameter("natbt_lo", [N, KT, 128], BF16,
                                         isOutput=False)
    ay0_d = nc.declare_dram_parameter("ayinit", [128, 2, N], BF16,
                                      isOutput=False)
    cf_d = nc.declare_dram_parameter("coefs", [128, unroll, half], F32,
                                     isOutput=False)
    xout_d = nc.declare_dram_parameter("xout", [128, KT, N], F32,
                                       isOutput=True)

    with tile.TileContext(nc) as tc:
        with (
            tc.tile_pool(name="weights", bufs=1) as wpool,
            tc.tile_pool(name="state", bufs=1) as spool,
            tc.tile_pool(name="tmp", bufs=2) as tpool,
            tc.tile_pool(name="psum_ay", bufs=2, space="PSUM") as ppool_ay,
            tc.tile_pool(name="psum_gp", bufs=1, space="PSUM") as ppool_gp,
        ):
            # --- persistent SBUF tensors -------------------------------
            at_hi = wpool.tile([128, KT, M], BF16)    # lhsT for matmul1
            at_lo = wpool.tile([128, KT, M], BF16)
            w2_hi = wpool.tile([128, KT, 128], BF16)  # lhsT for matmul2
            w2_lo = wpool.tile([128, KT, 128], BF16)
            # ay hi/lo pair packed in one tile: [:, 0, :] = hi, [:, 1, :] = lo
            # (one 48-col AP so matmul2 consumes both halves in one pass)
            ayp = wpool.tile([128, 2, N], BF16)
            cf_sb = wpool.tile([128, unroll, half], F32)  # momentum coefs
            # y kept fp32 plus its bf16 hi/lo split packed [128, G, 2, N]
            y_sb = [spool.tile([128, G0, N], F32, tag="y0", name="y0"),
                    spool.tile([128, G1, N], F32, tag="y1", name="y1")]
            yp_sb = [spool.tile([128, G0, 2, N], BF16, tag="yp0", name="yp0"),
                     spool.tile([128, G1, 2, N], BF16, tag="yp1", name="yp1")]
            # x ping-pong: x_sb[s][g]
            x_sb = [[spool.tile([128, G0, N], F32, tag=f"x{s}0", name=f"x{s}0"),
                     spool.tile([128, G1, N], F32, tag=f"x{s}1", name=f"x{s}1")]
                    for s in range(2)]

            nc.sync.dma_start(out=at_hi[:], in_=at_hi_d[:])
            nc.sync.dma_start(out=at_lo[:], in_=at_lo_d[:])
            nc.sync.dma_start(out=w2_hi[:], in_=a_hi_d[:])
            nc.sync.dma_start(out=w2_lo[:], in_=a_lo_d[:])
            # overwrite contraction rows CT0..CT0+N with the -Atb^T blocks
            nc.sync.dma_start(out=w2_hi[CT0:CT0 + N, :, :], in_=nat_hi_d[:])
            nc.sync.dma_start(out=w2_lo[CT0:CT0 + N, :, :], in_=nat_lo_d[:])
            nc.sync.dma_start(out=ayp[:], in_=ay0_d[:])
            nc.sync.dma_start(out=cf_sb[:], in_=cf_d[:])

            # initial state: x_prev = y = 0
            for g in range(2):
                nc.vector.memset(y_sb[g][:], 0.0)
                nc.vector.memset(yp_sb[g][:], 0.0)
                nc.vector.memset(x_sb[1][g][:], 0.0)

            # group g -> (ktile offset, ktile count)
            gidx = [(0, G0), (G0, G1)]

            def fista_step(cur, prev, coef_ap):
                do_mm = ablate != 'ew_only'
                do_ew = ablate != 'mm_only'
                # matmul1: Ay = A @ Y -> psum (81, 2*24): per ktile one
                # 48-col pass A_hi @ [y_hi|y_lo], plus A_lo @ y_hi
                # accumulated onto the left half. halves sum to the 3-term
                # split product.
                if do_mm:
                    ay_ps = ppool_ay.tile([M, 2, N], F32, tag="ay")
                    for kt in range(KT):
                        g, j = (0, kt) if kt < G0 else (1, kt - G0)
                        nc.tensor.matmul(
                            ay_ps[:],
                            at_hi[:, kt, :],
                            yp_sb[g][:, j, :, :],
                            start=(kt == 0),
                            stop=False,
                            skip_group_check=True,
                        )
                        nc.tensor.matmul(
                            ay_ps[:, 0, :],
                            at_lo[:, kt, :],
                            yp_sb[g][:, j, 0, :],
                            start=False,
                            stop=(kt == KT - 1),
                            skip_group_check=True,
                        )
                    # ay = left + right half (one PSUM operand per DVE op);
                    # split into bf16 hi + lo (identity rows CT0.. of ayp
                    # stay from init)
                    ayt = tpool.tile([M, N], F32, tag="ayt")
                    ays = tpool.tile([M, N], F32, tag="ays")
                    nc.vector.tensor_copy(ayt[:], ay_ps[:, 0, :])
                    nc.vector.scalar_tensor_tensor(
                        out=ays[:], in0=ay_ps[:, 1, :], scalar=1.0,
                        in1=ayt[:], op0=mybir.AluOpType.mult,
                        op1=mybir.AluOpType.add)
                    nc.vector.tensor_copy(ayp[0:M, 0, :], ays[:])
                    nc.vector.tensor_sub(ayp[0:M, 1, :], ays[:],
                                         ayp[0:M, 0, :])

                # matmul2 + elementwise, per column group. Gradient PSUM is
                # chunked into <=10-ktile tiles so a 48-col matmul output
                # never crosses a PSUM bank (512 fp32).
                for g in range(2):
                    k0, ng = gidx[g]
                    chunks = [(c0, min(10, ng - c0))
                              for c0 in range(0, ng, 10)]
                    z = tpool.tile([128, ng, N], F32, tag=f"z{g}")
                    if do_mm:
                        for c0, nch in chunks:
                            gp = ppool_gp.tile([128, nch, 2, N], F32,
                                               tag=f"gp{g}_{c0}",
                                               name=f"gp{g}_{c0}")
                            for j in range(nch):
                                nc.tensor.matmul(
                                    gp[:, j, :, :],
                                    w2_hi[:, k0 + c0 + j, :],
                                    ayp[:],
                                    start=True,
                                    stop=False,
                                    skip_group_check=True,
                                )
                                nc.tensor.matmul(
                                    gp[:, j, 0, :],
                                    w2_lo[:, k0 + c0 + j, :],
                                    ayp[:, 0, :],
                                    start=False,
                                    stop=True,
                                    skip_group_check=True,
                                )
                            # z = y - mu*(gpL + gpR): two chained stt ops,
                            # each reading one PSUM half
                            zc = z[:, c0:c0 + nch, :]
                            nc.vector.scalar_tensor_tensor(
                                out=zc, in0=gp[:, :, 0, :], scalar=-mu_s,
                                in1=y_sb[g][:, c0:c0 + nch, :],
                                op0=mybir.AluOpType.mult,
                                op1=mybir.AluOpType.add)
                            nc.vector.scalar_tensor_tensor(
                                out=zc, in0=gp[:, :, 1, :], scalar=-mu_s,
                                in1=zc, op0=mybir.AluOpType.mult,
                                op1=mybir.AluOpType.add)
                    else:
                        nc.vector.tensor_scalar_mul(z[:], y_sb[g][:], 0.5)
                    if chain == "pool" or (chain == "mixed" and g == 1):
                        eng = nc.gpsimd
                    else:
                        eng = nc.vector
                    # soft threshold: xn = z - clamp(z, -thr, thr)
                    c = tpool.tile([128, ng, N], F32, tag=f"c{g}")
                    eng.tensor_scalar(
                        out=c[:],
                        in0=z[:],
                        scalar1=thr,
                        scalar2=-thr,
                        op0=mybir.AluOpType.min,
                        op1=mybir.AluOpType.max,
                    )
                    xn = x_sb[cur][g]
                    eng.tensor_sub(xn[:], z[:], c[:])
                    # momentum: y = xn + coef*(xn - x_prev), then split y
                    d = tpool.tile([128, ng, N], F32, tag=f"d{g}")
                    eng.tensor_sub(d[:], xn[:], x_sb[prev][g][:])
                    # TensorScalarPtr with an AP scalar only runs on DVE
                    nc.vector.scalar_tensor_tensor(
                        out=y_sb[g][:],
                        in0=d[:],
                        scalar=coef_ap,
                        in1=xn[:],
                        op0=mybir.AluOpType.mult,
                        op1=mybir.AluOpType.add,
                    )
                    eng.tensor_copy(yp_sb[g][:, :, 0, :], y_sb[g][:])
                    eng.tensor_sub(yp_sb[g][:, :, 1, :], y_sb[g][:],
                                   yp_sb[g][:, :, 0, :])

            # --- FISTA iterations: hardware loop, `unroll` steps/body --
            with tc.For_i(0, half, 1,
                          hint_engines=((mybir.EngineType.PE,) if hint
                                        else ()),
                          staggered_reset=staggered) as it:
                for s in range(unroll):
                    fista_step(s % 2, (s + 1) % 2, cf_sb[:, s, ds(it, 1)])

            # --- write back final x ------------------------------------
            nc.sync.dma_start(out=xout_d[:, 0:G0, :], in_=x_sb[1][0][:])
            nc.sync.dma_start(out=xout_d[:, G0:KT, :], in_=x_sb[1][1][:])

    _legalize_waits(nc)
    return nc


def _split16(x):
    bf = mybir.dt.np(BF16)
    hi = x.astype(bf)
    lo = (x - hi.astype(np.float32)).astype(bf)
    return hi, lo


def _const_inputs(A, iters, unroll=2):
    """Input-independent operands: A^T tiles, A tiles, ay init, coefs."""
    if iters < unroll:
        unroll = iters
    A = np.asarray(A, np.float32)
    A_pad = np.zeros((M, DP), np.float32)
    A_pad[:, :D] = A
    a_tiles = np.zeros((128, KT, 128), np.float32)
    a_tiles[:M] = A_pad.reshape(M, KT, 128)
    ay_init = np.zeros((128, 2, N), np.float32)
    ay_init[CT0 : CT0 + N, 0] = np.eye(N, dtype=np.float32)
    at_tiles = np.ascontiguousarray(
        A_pad.T.reshape(KT, 128, M).transpose(1, 0, 2))  # [128, KT, M]

    coefs = _fista_coefs(iters)
    half = iters // unroll
    cf = np.zeros((unroll, half), np.float32)
    for s in range(unroll):
        cf[s] = coefs[s::unroll]
    cf_tab = np.ascontiguousarray(
        np.broadcast_to(cf[None], (128, unroll, half)).astype(np.float32))

    at_hi, at_lo = _split16(at_tiles)
    a_hi, a_lo = _split16(a_tiles)
    return {"at_hi": at_hi, "at_lo": at_lo, "a_hi": a_hi, "a_lo": a_lo,
            "ayinit": ay_init.astype(mybir.dt.np(BF16)),
            "coefs": cf_tab}, A_pad


def _natbt_inputs(inp, A_pad):
    """Per-core -Atb^T blocks (bf16 hi/lo), shaped [N, KT, 128]."""
    inp = np.asarray(inp, np.float32)
    his, los = [], []
    for c in range(NCORES):
        chunk = inp[c * BPC : (c + 1) * BPC]            # (8, 81, 3)
        b_mat = chunk.transpose(1, 0, 2).reshape(M, N)  # (81, 24)
        ct = b_mat.T @ A_pad                            # (24, 5248) = (Atb)^T
        hi, lo = _split16(np.ascontiguousarray(-ct.reshape(N, KT, 128)))
        his.append(hi)
        los.append(lo)
    return {"natbt_hi": his, "natbt_lo": los}


def _prep_inputs(inp, A, iters=None, unroll=2):
    """Per-core input maps (kept for compatibility with direct
    run_bass_kernel_spmd invocations, e.g. simulation)."""
    if iters is None:
        iters = ITERS
    const, A_pad = _const_inputs(A, iters, unroll)
    nat = _natbt_inputs(inp, A_pad)
    return [dict(const, natbt_hi=nat["natbt_hi"][c],
                 natbt_lo=nat["natbt_lo"][c]) for c in range(NCORES)]


class _Runner:
    """PJRT executable for one built module, jitted once. Input-independent
    operands live on device; per-call we upload only the -Atb^T blocks."""

    def __init__(self, nc, A, iters, unroll=2):
        import jax
        from jax.sharding import Mesh, PartitionSpec, NamedSharding
        from jax.experimental.shard_map import shard_map
        from concourse import bass2jax

        bass2jax.install_neuronx_cc_hook()
        self.nc = nc
        self.const, self.A_pad = _const_inputs(A, iters, unroll)

        assert nc.dbg_addr is None or not nc.dbg_callbacks
        extra = {}
        if nc.dbg_addr is not None:
            extra[nc.dbg_addr.name] = np.zeros((1, 2), np.uint32)

        partition_name = (nc.partition_id_tensor.name
                          if nc.partition_id_tensor else None)
        in_names, out_names, out_avals, zero_outs = [], [], [], []
        self.in_dtypes = {}
        for alloc in nc.m.functions[0].allocations:
            if not isinstance(alloc, mybir.MemoryLocationSet):
                continue
            name = alloc.memorylocations[0].name
            if alloc.kind == "ExternalInput":
                if name != partition_name:
                    in_names.append(name)
                    self.in_dtypes[name] = mybir.dt.np(alloc.dtype)
            elif alloc.kind == "ExternalOutput":
                shape = tuple(alloc.tensor_shape)
                dtype = mybir.dt.np(alloc.dtype)
                out_names.append(name)
                out_avals.append(jax.core.ShapedArray(shape, dtype))
                zero_outs.append(np.zeros(shape, dtype))
        n_params = len(in_names)
        n_outs = len(out_names)
        all_names = in_names + out_names
        if partition_name is not None:
            all_names.append(partition_name)

        def _body(*args):
            operands = list(args)
            if partition_name is not None:
                operands.append(bass2jax.partition_id_tensor())
            outs = bass2jax._bass_exec_p.bind(
                *operands,
                out_avals=tuple(out_avals),
                in_names=tuple(all_names),
                out_names=tuple(out_names),
                lowering_input_output_aliases=(),
                sim_require_finite=True,
                sim_require_nnan=True,
                nc=nc,
            )
            return tuple(outs)

        devices = jax.devices()[:NCORES]
        assert len(devices) == NCORES
        self.mesh = Mesh(np.asarray(devices), ("core",))
        in_specs = (PartitionSpec("core"),) * (n_params + n_outs)
        out_specs = (PartitionSpec("core"),) * n_outs
        donate = tuple(range(n_params, n_params + n_outs))
        self.fn = jax.jit(
            shard_map(_body, mesh=self.mesh, in_specs=in_specs,
                      out_specs=out_specs, check_rep=False),
            donate_argnums=donate, keep_unused=True)

        self.in_names = in_names
        self.out_names = out_names
        self.out_avals = out_avals
        self.zero_shapes = [(z.shape, z.dtype) for z in zero_outs]

        # Device-resident constant inputs (replicated per core, concat on
        # axis 0 as shard_map expects). extra covers dbg_addr if present.
        sharding = NamedSharding(self.mesh, PartitionSpec("core"))
        self.dev_const = {}
        for name in in_names:
            if name.startswith("natbt"):
                continue
            arr = self.const.get(name)
            if arr is None:
                arr = extra[name]
            glob = np.concatenate([arr] * NCORES, axis=0).astype(
                self.in_dtypes[name])
            self.dev_const[name] = jax.device_put(glob, sharding)

    def prep(self, inp):
        """Host-side per-call input prep (the -Atb^T blocks)."""
        nat = _natbt_inputs(inp, self.A_pad)
        return {name: np.concatenate(nat[name], axis=0).astype(
                    self.in_dtypes[name])
                for name in self.in_names if name.startswith("natbt")}

    def run_prepped(self, prepped):
        args = []
        for name in self.in_names:
            if name.startswith("natbt"):
                args.append(prepped[name])
            else:
                args.append(self.dev_const[name])
        for shape, dtype in self.zero_shapes:
            args.append(np.zeros((NCORES * shape[0], *shape[1:]), dtype))
        out_arrs = self.fn(*args)
        res = []
        for c in range(NCORES):
            res.append({
                name: np.asarray(out_arrs[i]).reshape(
                    NCORES, *self.out_avals[i].shape)[c]
                for i, name in enumerate(self.out_names)})
        return res

    def run(self, inp):
        return self.run_prepped(self.prep(inp))


def _unshard(results):
    outs = []
    for c in range(NCORES):
        xo = np.asarray(results[c]["xout"])              # [128, KT, N]
        x_dn = xo.transpose(1, 0, 2).reshape(DP, N)[:D]  # (5184, 24)
        outs.append(x_dn.reshape(72, 72, BPC, 3).transpose(2, 0, 1, 3))
    return np.concatenate(outs, 0).astype(np.float32)    # (64, 72, 72, 3)


def _get_runner(A, mu_s, thr, iters):
    key = (mu_s, thr, iters, hash(np.asarray(A, np.float32).tobytes()))
    if key not in _RUNNERS:
        nc = _build(mu_s, thr, iters)
        _RUNNERS[key] = _Runner(nc, A, iters)
    return _RUNNERS[key]


def _run(inp, A, lam, mu, trace=False):
    mu_s = float(np.asarray(mu).reshape(-1)[0])
    thr = float(np.asarray(lam).reshape(-1)[0]) * mu_s
    runner = _get_runner(A, mu_s, thr, ITERS)
    results = runner.run(inp)
    return _unshard(results), results


def kernel(inp, A, lam, mu):
    out, _ = _run(inp, A, lam, mu)
    return out
